# revision 1
# baseline (speedup 1.0000x reference)
"""Edge-conditioned causal self-attention (Graphormer-style) on 8 Trainium2 cores.

Math (per batch b, head h; i = query pos, j = key pos; E=32, G=16 edge types):
  scores[i,j] = sum_e q[i,e] * k[j,e] * ektab[bm[j,i], e] / sqrt(E)
  attn        = softmax_j(scores + biastab[bm[i,j]])  (causal j<=i)
  y[i,e]      = sum_j attn[i,j] * v[j,e] * evtab[bm[j,i], e]
  out         = y @ w_proj
where ektab = edge_emb @ w_edge_k (per-head slice), evtab likewise, and bm is
the integer edge-type matrix.  NOTE: the additive bias indexes bm TRANSPOSED
relative to the k/v modulation (matches the reference einsums).

Because there are only G=16 edge types, scores decompose into 16 per-type Gram
matrices S_g = (q .* ektab[g]) @ k^T that never materialize the (B,T,T,C)
gather.  Since the type masks partition the (j,i) plane, exp() commutes with
masking, so the per-type masked softmax numerators are
  E_g[j,i] = (bm[j,i]==g) * exp(S_g[j,i]) * exp(biastab[bm[i,j]])
and the output accumulates yT += (v .* evtab[g] | 1)-contractions of E_g,
with the ones column producing the softmax denominator for free.  Planes are
(j-chunk 128, i in [128*jc, T)) — causal triangularity skips the dead i-range.

Sharding: core c -> batch b=c//4, head pair (2*(c%4), 2*(c%4)+1).  Fully
data-parallel, no collectives; host sums the 4 per-core partial projections
per batch (w_proj is row-parallel over head slices).
"""

import numpy as np

import concourse.bass as bass  # noqa: F401
import concourse.mybir as mybir
import concourse.tile as tile
from concourse import bacc
from concourse.bass_utils import run_bass_kernel_spmd

B, T, C, H, E, G = 2, 512, 256, 8, 32, 16
NCORES = 8
EP = E + 1      # 33: v block width per head (32 modulated v cols + ones col)
VW = 2 * EP     # 66: v block for both heads
TC = T // 128   # 4 row chunks

F16 = mybir.dt.float16
F32 = mybir.dt.float32
EXP = mybir.ActivationFunctionType.Exp
EQ = mybir.AluOpType.is_equal
MUL = mybir.AluOpType.mult

_NC_CACHE = [None]


def _build_nc():
    nc = bacc.Bacc("TRN2", target_bir_lowering=False)

    d_xT = nc.dram_tensor("xT", (C, T), F32, kind="ExternalInput")
    d_mm = nc.dram_tensor("mm", (T, T), F16, kind="ExternalInput")
    d_cb = nc.dram_tensor("cb", (2 * T, T), F16, kind="ExternalInput")
    d_wqk = nc.dram_tensor("w_qk", (C, 128), F32, kind="ExternalInput")
    d_wv = nc.dram_tensor("w_v", (C, 64), F32, kind="ExternalInput")
    d_ekT = nc.dram_tensor("ektabT", (64, G), F32, kind="ExternalInput")
    d_evb = nc.dram_tensor("evb", (128, G * VW), F16, kind="ExternalInput")
    d_wp = nc.dram_tensor("w_proj_sl", (64, C), F16, kind="ExternalInput")
    d_out = nc.dram_tensor("out", (T, C), F32, kind="ExternalOutput")

    with tile.TileContext(nc) as tc:
        with (
            tc.tile_pool(name="const", bufs=1) as const,
            tc.tile_pool(name="ps_misc", bufs=2, space="PSUM") as ps_misc,
            tc.tile_pool(name="ps_s", bufs=4, space="PSUM") as ps_s,
            tc.tile_pool(name="ps_y", bufs=2, space="PSUM") as ps_y,
            tc.tile_pool(name="expp", bufs=6) as expp,
            tc.tile_pool(name="e1p", bufs=6) as e1p,
            tc.tile_pool(name="e2p", bufs=6) as e2p,
        ):
            # ---------------- input loads ----------------
            xT_s = []
            for i in range(2):
                t_ = const.tile([128, T], F32, name=f"xT{i}")
                nc.sync.dma_start(out=t_[:], in_=d_xT[128 * i:128 * (i + 1), :])
                xT_s.append(t_)
            wqk_s = []
            for i in range(2):
                t_ = const.tile([128, 128], F32, name=f"wqk{i}")
                nc.sync.dma_start(out=t_[:], in_=d_wqk[128 * i:128 * (i + 1), :])
                wqk_s.append(t_)
            wv_s = []
            for i in range(2):
                t_ = const.tile([128, 64], F32, name=f"wv{i}")
                nc.sync.dma_start(out=t_[:], in_=d_wv[128 * i:128 * (i + 1), :])
                wv_s.append(t_)
            ekT_s = const.tile([64, G], F32, name="ekT")
            nc.sync.dma_start(out=ekT_s[:], in_=d_ekT[:, :])
            mm_s = []
            for j in range(TC):
                t_ = const.tile([128, T], F16, name=f"mm{j}")
                nc.sync.dma_start(out=t_[:], in_=d_mm[128 * j:128 * (j + 1), :])
                mm_s.append(t_)
            cb_s = [[], []]
            for h in range(2):
                for j in range(TC):
                    t_ = const.tile([128, T], F16, name=f"cb{h}_{j}")
                    nc.sync.dma_start(
                        out=t_[:], in_=d_cb[h * T + 128 * j:h * T + 128 * (j + 1), :])
                    cb_s[h].append(t_)
            evb_s = const.tile([128, G * VW], F16, name="evb")
            nc.sync.dma_start(out=evb_s[:], in_=d_evb[:, :])
            wp_s = const.tile([64, C], F16, name="wp")
            nc.sync.dma_start(out=wp_s[:], in_=d_wp[:, :])

            # ---------------- q/k/v projections ----------------
            # qk_ps rows: 0-31 qT(h0), 32-63 qT(h1), 64-95 kT(h0), 96-127 kT(h1)
            qk_ps = ps_misc.tile([128, T], F32, tag="misc", name="qk_ps")
            for ck in range(2):
                nc.tensor.matmul(
                    qk_ps[:],
                    lhsT=wqk_s[ck][:],
                    rhs=xT_s[ck][:],
                    start=(ck == 0), stop=(ck == 1),
                )
            # v_ps cols per t-chunk: 32 v(h0) | 32 v(h1)
            v_ps = ps_misc.tile([128, TC * 64], F32, tag="misc", name="v_ps")
            for tcc in range(TC):
                for ck in range(2):
                    nc.tensor.matmul(
                        v_ps[:, 64 * tcc:64 * (tcc + 1)],
                        lhsT=xT_s[ck][:, 128 * tcc:128 * (tcc + 1)],
                        rhs=wv_s[ck][:],
                        start=(ck == 0), stop=(ck == 1),
                    )

            qk_sb = const.tile([128, T], F16, name="qk_sb")
            nc.vector.tensor_copy(out=qk_sb[:], in_=qk_ps[:])

            # k tiles (contraction layout: e on partitions, j on free)
            k_h = []
            for h in range(2):
                t_ = const.tile([E, T], F16, name=f"k{h}")
                nc.vector.tensor_copy(out=t_[:], in_=qk_sb[64 + 32 * h:96 + 32 * h, :])
                k_h.append(t_)

            # q' tiles: per g block of T cols, rows = e, modulated by ektab/sqrt(E)
            q_all = []
            for h in range(2):
                t_ = const.tile([E, G * T], F16, name=f"qall{h}")
                for g in range(G):
                    eng = nc.vector if (g % 2 == 0) else nc.gpsimd
                    eng.tensor_scalar_mul(
                        t_[:, T * g:T * (g + 1)],
                        qk_sb[32 * h:32 * h + 32, :],
                        ekT_s[32 * h:32 * h + 32, g:g + 1],
                    )
                q_all.append(t_)

            # v66 per t-chunk: [v_h0(32) | 1 | v_h1(32) | 1]
            v66 = []
            for tcc in range(TC):
                t_ = const.tile([128, VW], F16, name=f"v66_{tcc}")
                nc.vector.tensor_copy(out=t_[:, 0:32], in_=v_ps[:, 64 * tcc:64 * tcc + 32])
                nc.vector.tensor_copy(out=t_[:, 33:65], in_=v_ps[:, 64 * tcc + 32:64 * tcc + 64])
                nc.vector.memset(t_[:, 32:33], 1.0)
                nc.vector.memset(t_[:, 65:66], 1.0)
                v66.append(t_)

            # v'_g = v66 * evtab broadcast (both heads + ones cols in one op)
            vall = []
            for tcc in range(TC):
                t_ = const.tile([128, G * VW], F16, name=f"vall{tcc}")
                for g in range(G):
                    eng = nc.gpsimd if (g % 2 == 0) else nc.vector
                    eng.tensor_mul(
                        t_[:, VW * g:VW * (g + 1)],
                        v66[tcc][:],
                        evb_s[:, VW * g:VW * (g + 1)],
                    )
                vall.append(t_)

            ones32 = const.tile([1, 32], F32, name="ones32")
            nc.vector.memset(ones32[:], 1.0)
            yTn = const.tile([64, T], F16, name="yTn")

            # ---------------- main attention loops ----------------
            for h in range(2):
                yT = ps_y.tile([EP, T], F32, tag="y", name=f"yT{h}")
                first = True
                for jc in range(TC):
                    s0 = 128 * jc
                    fi = T - s0
                    for g in range(G):
                        s_ps = ps_s.tile([128, fi], F32, tag="s", name="s_ps")
                        nc.tensor.matmul(
                            s_ps[:],
                            lhsT=k_h[h][:, s0:s0 + 128],
                            rhs=q_all[h][:, T * g + s0:T * (g + 1)],
                            start=True, stop=True,
                        )
                        p_sb = expp.tile([128, fi], F16, tag="p", name="p_sb")
                        nc.scalar.activation(p_sb[:], s_ps[:], EXP)
                        e1 = e1p.tile([128, fi], F16, tag="e1", name="e1")
                        nc.vector.scalar_tensor_tensor(
                            out=e1[:],
                            in0=mm_s[jc][:, s0:],
                            scalar=float(g),
                            in1=p_sb[:],
                            op0=EQ, op1=MUL,
                        )
                        e2 = e2p.tile([128, fi], F16, tag="e2", name="e2")
                        cb_eng = nc.vector if ((h * 64 + jc * G + g) % 5 == 4) else nc.gpsimd
                        cb_eng.tensor_mul(e2[:], e1[:], cb_s[h][jc][:, s0:])
                        nc.tensor.matmul(
                            yT[:, s0:],
                            lhsT=vall[jc][:, VW * g + EP * h:VW * g + EP * (h + 1)],
                            rhs=e2[:],
                            start=first, stop=(jc == TC - 1 and g == G - 1),
                        )
                        first = False

                # normalize: y[:, i] *= 1/rowsum[i]  (rowsum = yT row 32)
                recip = const.tile([1, T], F32, name=f"recip{h}")
                nc.vector.reciprocal(recip[:], yT[32:33, :])
                bc_ps = ps_misc.tile([32, T], F32, tag="misc", name=f"bc{h}")
                nc.tensor.matmul(bc_ps[:], lhsT=ones32[:], rhs=recip[:],
                                 start=True, stop=True)
                rb_sb = const.tile([32, T], F32, name=f"rb{h}")
                nc.scalar.copy(rb_sb[:], bc_ps[:])
                nc.vector.tensor_mul(yTn[32 * h:32 * (h + 1), :], yT[0:32, :], rb_sb[:])

            # ---------------- output projection ----------------
            for tcc in range(TC):
                o_ps = ps_misc.tile([128, C], F32, tag="misc", name=f"o_ps{tcc}")
                nc.tensor.matmul(
                    o_ps[:],
                    lhsT=yTn[:, 128 * tcc:128 * (tcc + 1)],
                    rhs=wp_s[:],
                    start=True, stop=True,
                )
                o_sb = const.tile([128, C], F32, name=f"o_sb{tcc}")
                nc.scalar.copy(o_sb[:], o_ps[:])
                nc.sync.dma_start(out=d_out[128 * tcc:128 * (tcc + 1), :], in_=o_sb[:])

    nc.compile()
    return nc


def _get_nc():
    if _NC_CACHE[0] is None:
        _NC_CACHE[0] = _build_nc()
    return _NC_CACHE[0]


def _prep_core_inputs(c, x, bm, w_attn, w_proj, w_edge_k, w_edge_v, eet, abt):
    b, hp = divmod(c, 4)
    h0 = 2 * hp
    xT = np.ascontiguousarray(x[b].T).astype(np.float32)            # (C, T)
    triu = np.triu(np.ones((T, T), dtype=bool))                     # j <= i
    mm = np.where(triu, bm[b], 255).astype(np.float16)              # (T, T) [j,i]
    w_qk = np.concatenate(
        [w_attn[:, hp * 64:(hp + 1) * 64],
         w_attn[:, C + hp * 64:C + (hp + 1) * 64]], axis=1
    ).astype(np.float32)                                            # (C, 128)
    w_v = np.ascontiguousarray(
        w_attn[:, 2 * C + hp * 64:2 * C + (hp + 1) * 64]).astype(np.float32)
    ektab = (eet @ w_edge_k)[:, hp * 64:(hp + 1) * 64] / np.sqrt(E)  # (G, 64)
    ektabT = np.ascontiguousarray(ektab.T).astype(np.float32)        # (64, G)
    evtab = (eet @ w_edge_v)[:, hp * 64:(hp + 1) * 64]               # (G, 64)
    evb = np.zeros((128, G * VW), np.float16)
    for g in range(G):
        evb[:, VW * g:VW * g + 32] = evtab[g, 0:32].astype(np.float16)[None, :]
        evb[:, VW * g + 32] = 1.0
        evb[:, VW * g + 33:VW * g + 65] = evtab[g, 32:64].astype(np.float16)[None, :]
        evb[:, VW * g + 65] = 1.0
    ebias = np.exp(abt)                                              # (G, H)
    cb = np.empty((2 * T, T), np.float16)
    bmT = bm[b].T                                                    # bmT[j,i] = bm[b][i,j]
    for h in range(2):
        cb[h * T:(h + 1) * T, :] = ebias[:, h0 + h][bmT]
    w_proj_sl = np.ascontiguousarray(
        w_proj[hp * 64:(hp + 1) * 64, :]).astype(np.float16)         # (64, C)
    return {
        "xT": xT, "mm": mm, "cb": cb, "w_qk": w_qk, "w_v": w_v,
        "ektabT": ektabT, "evb": evb, "w_proj_sl": w_proj_sl,
    }


def run(inputs, trace=False):
    x = np.asarray(inputs["x"], np.float32)
    bm = np.asarray(inputs["bias_matrix"]).astype(np.int64)
    w_attn = np.asarray(inputs["w_attn"], np.float32)
    w_proj = np.asarray(inputs["w_proj"], np.float32)
    w_edge_k = np.asarray(inputs["w_edge_k"], np.float32)
    w_edge_v = np.asarray(inputs["w_edge_v"], np.float32)
    eet = np.asarray(inputs["edge_emb_table"], np.float32)
    abt = np.asarray(inputs["attn_bias_table"], np.float32)

    nc = _get_nc()
    in_maps = [
        _prep_core_inputs(c, x, bm, w_attn, w_proj, w_edge_k, w_edge_v, eet, abt)
        for c in range(NCORES)
    ]
    res = run_bass_kernel_spmd(nc, in_maps, core_ids=list(range(NCORES)),
                               trace=trace)
    out = np.zeros((B, T, C), np.float32)
    for c in range(NCORES):
        out[c // 4] += res.results[c]["out"]
    return out, res


def kernel(**inputs) -> np.ndarray:
    out, _ = run(inputs, trace=False)
    return out



# revision 12
# speedup vs baseline: 1.1007x; 1.1007x over previous
"""Edge-conditioned causal self-attention (Graphormer-style) on 8 Trainium2 cores.

Math (per batch b, head h; i = query pos, j = key pos; E=32, G=16 edge types):
  scores[i,j] = sum_e q[i,e] * k[j,e] * ektab[bm[j,i], e] / sqrt(E)
  attn        = softmax_j(scores + biastab[bm[i,j]])  (causal j<=i)
  y[i,e]      = sum_j attn[i,j] * v[j,e] * evtab[bm[j,i], e]
  out         = y @ w_proj
where ektab = edge_emb @ w_edge_k (per-head slice), evtab likewise, and bm is
the integer edge-type matrix.  NOTE: the additive bias indexes bm TRANSPOSED
relative to the k/v modulation (matches the reference einsums).

Since there are only G=16 edge types, scores decompose into 16 per-type Gram
planes S_g = (q .* ektab[g]) @ k^T.  The per-(j,i) plane selection is done by
masking in exp space: e2_g = mask_g * exp(S_g) * exp(bias), and the output
accumulates yT += (v .* evtab[g] | 1) @ e2_g with the ones column giving the
softmax denominator for free.

v2 structure (engine-balanced for the TimelineSim cost model):
  - S_g matmuls batched into multi-bank PSUM tiles; exp'd in batched strided
    activations on the Scalar engine (only engine with cheap PSUM exit).
  - mask multiply (e1): bottom NG g's per (h,jc) on GPSIMD via fused
    scalar_tensor_tensor (mm==g)*p (needs no mask planes); top NV g's on
    Vector via plain tensor_tensor against host-precomputed f16 mask planes
    (TT runs at 2x, STT only at 1x in the cost model).
  - bias multiply (e2): on Vector as wide TTs with stride-0-broadcast cb.
Sharding: core c -> batch b=c//4, head pair (2*(c%4), 2*(c%4)+1).  Fully
data-parallel, no collectives; host sums the 4 per-core partial projections
per batch (w_proj is row-parallel over head slices).
"""

import numpy as np

import concourse.bass as bass  # noqa: F401
import concourse.mybir as mybir
import concourse.tile as tile
from concourse import bacc
from concourse.bass_utils import run_bass_kernel_spmd

B, T, C, H, E, G = 2, 512, 256, 8, 32, 16
NCORES = 8
EP = E + 1      # 33: v block width per head (32 modulated v cols + ones col)
VW = 2 * EP     # 66: v block for both heads
TC = T // 128   # 4 row chunks

NG = 4          # bottom-NG g's per (h,jc) handled on GPSIMD (e1+e2 TTs)
FI = [512, 384, 256, 128]          # i-extent per j-chunk
PSTRIDE = [512, 512, 256, 128]     # psum plane stride per j-chunk
# per-jc PSUM batching: list of (g0, nplanes) with nplanes*PSTRIDE <= 1536
BATCHES = [
    [(0, 3), (3, 3), (6, 3), (9, 3), (12, 3), (15, 1)],
    [(0, 3), (3, 3), (6, 3), (9, 3), (12, 3), (15, 1)],
    [(0, 6), (6, 6), (12, 4)],
    [(0, 12), (12, 4)],
]

F16 = mybir.dt.float16
F32 = mybir.dt.float32
EXP = mybir.ActivationFunctionType.Exp
EQ = mybir.AluOpType.is_equal
MUL = mybir.AluOpType.mult

_NC_CACHE = [None]


def _build_nc():
    nc = bacc.Bacc("TRN2", target_bir_lowering=False)

    d_xT = nc.dram_tensor("xT", (C, T), F32, kind="ExternalInput")
    d_msk = [
        nc.dram_tensor(f"msk{jc}", (128, G * FI[jc]), F16, kind="ExternalInput")
        for jc in range(TC)
    ]
    d_cb = nc.dram_tensor("cb", (2 * T, T), F16, kind="ExternalInput")
    d_wqk = nc.dram_tensor("w_qk", (C, 128), F32, kind="ExternalInput")
    d_wv = nc.dram_tensor("w_v", (C, 64), F32, kind="ExternalInput")
    d_ekT = nc.dram_tensor("ektabT", (64, G), F32, kind="ExternalInput")
    d_evb = nc.dram_tensor("evb", (128, G * VW), F16, kind="ExternalInput")
    d_wp = nc.dram_tensor("w_proj_sl", (64, C), F16, kind="ExternalInput")
    d_out = nc.dram_tensor("out", (T, C), F32, kind="ExternalOutput")

    with tile.TileContext(nc) as tc:
        with (
            tc.tile_pool(name="const", bufs=1) as const,
            tc.tile_pool(name="ps_misc", bufs=1, space="PSUM") as ps_misc,
            tc.tile_pool(name="ps_tri", bufs=2, space="PSUM") as ps_tri,
            tc.tile_pool(name="ps_y", bufs=1, space="PSUM") as ps_y,
            tc.tile_pool(name="pp", bufs=3) as pp,
            tc.tile_pool(name="e1p", bufs=2) as e1p,
            tc.tile_pool(name="e2p", bufs=2) as e2p,
        ):
            # ---------------- input loads ----------------
            xT_s = []
            for i in range(2):
                t_ = const.tile([128, T], F32, name=f"xT{i}")
                nc.sync.dma_start(out=t_[:], in_=d_xT[128 * i:128 * (i + 1), :])
                xT_s.append(t_)
            wqk_s = []
            for i in range(2):
                t_ = const.tile([128, 128], F32, name=f"wqk{i}")
                nc.sync.dma_start(out=t_[:], in_=d_wqk[128 * i:128 * (i + 1), :])
                wqk_s.append(t_)
            wv_s = []
            for i in range(2):
                t_ = const.tile([128, 64], F32, name=f"wv{i}")
                nc.sync.dma_start(out=t_[:], in_=d_wv[128 * i:128 * (i + 1), :])
                wv_s.append(t_)
            ekT_s = const.tile([64, G], F32, name="ekT")
            nc.sync.dma_start(out=ekT_s[:], in_=d_ekT[:, :])
            # per-jc tiles, in processing order: masks (e1), cb (bias; h0 first)
            msk_s = []
            cb_s = [[None] * TC, [None] * TC]
            for jc in range(TC):
                t_ = const.tile([128, G * FI[jc]], F16, name=f"msk{jc}")
                nc.sync.dma_start(out=t_[:], in_=d_msk[jc][:, :])
                msk_s.append(t_)
                t_ = const.tile([128, T], F16, name=f"cb0_{jc}")
                nc.sync.dma_start(
                    out=t_[:], in_=d_cb[128 * jc:128 * (jc + 1), :])
                cb_s[0][jc] = t_
            evb_s = const.tile([128, G * VW], F16, name="evb")
            nc.sync.dma_start(out=evb_s[:], in_=d_evb[:, :])
            for jc in range(TC):
                t_ = const.tile([128, T], F16, name=f"cb1_{jc}")
                nc.sync.dma_start(
                    out=t_[:], in_=d_cb[T + 128 * jc:T + 128 * (jc + 1), :])
                cb_s[1][jc] = t_
            wp_s = const.tile([64, C], F16, name="wp")
            nc.sync.dma_start(out=wp_s[:], in_=d_wp[:, :])

            # ---------------- q/k/v projections ----------------
            # qk_ps rows: 0-31 qT(h0), 32-63 qT(h1), 64-95 kT(h0), 96-127 kT(h1)
            qk_ps = ps_misc.tile([128, T], F32, tag="misc", name="qk_ps")
            for ck in range(2):
                nc.tensor.matmul(
                    qk_ps[:],
                    lhsT=wqk_s[ck][:],
                    rhs=xT_s[ck][:],
                    start=(ck == 0), stop=(ck == 1),
                )
            qk_sb = const.tile([128, T], F16, name="qk_sb")
            nc.vector.tensor_copy(out=qk_sb[:], in_=qk_ps[:])

            # v_ps cols per t-chunk: 32 v(h0) | 32 v(h1)
            v_ps = ps_misc.tile([128, TC * 64], F32, tag="misc", name="v_ps")
            for tcc in range(TC):
                for ck in range(2):
                    nc.tensor.matmul(
                        v_ps[:, 64 * tcc:64 * (tcc + 1)],
                        lhsT=xT_s[ck][:, 128 * tcc:128 * (tcc + 1)],
                        rhs=wv_s[ck][:],
                        start=(ck == 0), stop=(ck == 1),
                    )

            # k tiles (contraction layout: e on partitions, j on free)
            k_h = []
            for h in range(2):
                t_ = const.tile([E, T], F16, name=f"k{h}")
                nc.vector.tensor_copy(out=t_[:], in_=qk_sb[64 + 32 * h:96 + 32 * h, :])
                k_h.append(t_)

            # q' tiles: per g block of T cols, rows = e, modulated by ektab/sqrt(E)
            q_all = []
            for h in range(2):
                t_ = const.tile([E, G * T], F16, name=f"qall{h}")
                for g in range(G):
                    nc.vector.tensor_scalar_mul(
                        t_[:, T * g:T * (g + 1)],
                        qk_sb[32 * h:32 * h + 32, :],
                        ekT_s[32 * h:32 * h + 32, g:g + 1],
                    )
                q_all.append(t_)

            # v66 per t-chunk: [v_h0(32) | 1 | v_h1(32) | 1]
            v66 = []
            for tcc in range(TC):
                t_ = const.tile([128, VW], F16, name=f"v66_{tcc}")
                nc.vector.tensor_copy(out=t_[:, 0:32], in_=v_ps[:, 64 * tcc:64 * tcc + 32])
                nc.vector.tensor_copy(out=t_[:, 33:65], in_=v_ps[:, 64 * tcc + 32:64 * tcc + 64])
                nc.vector.memset(t_[:, 32:33], 1.0)
                nc.vector.memset(t_[:, 65:66], 1.0)
                v66.append(t_)

            # v'_g = v66 * evtab broadcast: one wide TT per t-chunk
            vall = []
            for tcc in range(TC):
                t_ = const.tile([128, G * VW], F16, name=f"vall{tcc}")
                nc.vector.tensor_mul(
                    t_[:].rearrange("p (g w) -> p g w", g=G),
                    v66[tcc][:].unsqueeze(1).broadcast_to([128, G, VW]),
                    evb_s[:].rearrange("p (g w) -> p g w", g=G),
                )
                vall.append(t_)

            ones32 = const.tile([1, 32], F32, name="ones32")
            nc.vector.memset(ones32[:], 1.0)
            yTn = const.tile([64, T], F16, name="yTn")

            # ---------------- main attention loops ----------------
            for h in range(2):
                yT = ps_y.tile([EP, T], F32, tag="y", name=f"yT{h}")
                first = True
                for jc in range(TC):
                    s0 = 128 * jc
                    fi = FI[jc]
                    ps = PSTRIDE[jc]
                    e1t = e1p.tile([128, G * fi], F16, tag="e1", name="e1")
                    e2t = e2p.tile([128, G * fi], F16, tag="e2", name="e2")
                    for (g0, np_) in BATCHES[jc]:
                        ghi = g0 + np_
                        tri = ps_tri.tile([128, 1536], F32, tag="tri", name="tri")
                        for k in range(np_):
                            g = g0 + k
                            nc.tensor.matmul(
                                tri[:, ps * k:ps * k + fi],
                                lhsT=k_h[h][:, s0:s0 + 128],
                                rhs=q_all[h][:, T * g + s0:T * (g + 1)],
                                start=True, stop=True,
                            )
                        p_ = pp.tile([128, 1536], F16, tag="p", name="p")
                        if ps == fi:
                            nc.scalar.activation(
                                p_[:, 0:np_ * fi], tri[:, 0:np_ * fi], EXP)
                        else:
                            nc.scalar.activation(
                                p_[:, 0:np_ * ps].rearrange(
                                    "p (k s) -> p k s", k=np_)[:, :, 0:fi],
                                tri[:, 0:np_ * ps].rearrange(
                                    "p (k s) -> p k s", k=np_)[:, :, 0:fi],
                                EXP,
                            )

                        # e1 = mask * p; e2 = e1 * cb.  g < NG on gpsimd
                        # (per-plane TTs), g >= NG on vector (wide TTs).
                        for k in range(np_):
                            g = g0 + k
                            if g >= NG:
                                break
                            nc.gpsimd.tensor_mul(
                                e1t[:, g * fi:(g + 1) * fi],
                                msk_s[jc][:, g * fi:(g + 1) * fi],
                                p_[:, ps * k:ps * k + fi],
                            )
                            nc.gpsimd.tensor_mul(
                                e2t[:, g * fi:(g + 1) * fi],
                                e1t[:, g * fi:(g + 1) * fi],
                                cb_s[h][jc][:, s0:s0 + fi],
                            )
                        glo = max(g0, NG)
                        if ghi > glo:
                            nrun = ghi - glo
                            koff = glo - g0
                            psrc = p_[:, ps * koff:ps * (koff + nrun)].rearrange(
                                "p (k s) -> p k s", k=nrun)[:, :, 0:fi]
                            nc.vector.tensor_mul(
                                e1t[:, glo * fi:ghi * fi].rearrange(
                                    "p (k s) -> p k s", k=nrun),
                                msk_s[jc][:, glo * fi:ghi * fi].rearrange(
                                    "p (k s) -> p k s", k=nrun),
                                psrc,
                            )
                            nc.vector.tensor_mul(
                                e2t[:, glo * fi:ghi * fi].rearrange(
                                    "p (k s) -> p k s", k=nrun),
                                e1t[:, glo * fi:ghi * fi].rearrange(
                                    "p (k s) -> p k s", k=nrun),
                                cb_s[h][jc][:, s0:s0 + fi].unsqueeze(1).broadcast_to(
                                    [128, nrun, fi]),
                            )

                        # yT accumulation
                        for k in range(np_):
                            g = g0 + k
                            nc.tensor.matmul(
                                yT[:, s0:s0 + fi],
                                lhsT=vall[jc][:, VW * g + EP * h:VW * g + EP * (h + 1)],
                                rhs=e2t[:, g * fi:(g + 1) * fi],
                                start=first,
                                stop=(jc == TC - 1 and g == G - 1),
                            )
                            first = False

                # normalize: y[:, i] *= 1/rowsum[i]  (rowsum = yT row 32)
                recip = const.tile([1, T], F32, name=f"recip{h}")
                nc.vector.reciprocal(recip[:], yT[32:33, :])
                bc_ps = ps_misc.tile([32, T], F32, tag="misc", name=f"bc{h}")
                nc.tensor.matmul(bc_ps[:], lhsT=ones32[:], rhs=recip[:],
                                 start=True, stop=True)
                rb_sb = const.tile([32, T], F32, name=f"rb{h}")
                nc.scalar.copy(rb_sb[:], bc_ps[:])
                nc.vector.tensor_mul(yTn[32 * h:32 * (h + 1), :], yT[0:32, :], rb_sb[:])

            # ---------------- output projection ----------------
            for tcc in range(TC):
                o_ps = ps_misc.tile([128, C], F32, tag="misc", name=f"o_ps{tcc}")
                nc.tensor.matmul(
                    o_ps[:],
                    lhsT=yTn[:, 128 * tcc:128 * (tcc + 1)],
                    rhs=wp_s[:],
                    start=True, stop=True,
                )
                o_sb = const.tile([128, C], F32, name=f"o_sb{tcc}")
                nc.scalar.copy(o_sb[:], o_ps[:])
                nc.sync.dma_start(out=d_out[128 * tcc:128 * (tcc + 1), :], in_=o_sb[:])

    nc.compile()
    return nc


def _get_nc():
    if _NC_CACHE[0] is None:
        _NC_CACHE[0] = _build_nc()
    return _NC_CACHE[0]


def _prep_core_inputs(c, x, bm, w_attn, w_proj, w_edge_k, w_edge_v, eet, abt):
    b, hp = divmod(c, 4)
    h0 = 2 * hp
    xT = np.ascontiguousarray(x[b].T).astype(np.float32)            # (C, T)
    triu = np.triu(np.ones((T, T), dtype=bool))                     # j <= i
    bmT = bm[b].T                                                   # bmT[j,i] = bm[b][i,j]
    mm = np.where(triu, bm[b], 255)                                 # (T, T) [j,i]
    # mask planes for all G edge types, per j-chunk, [j, g, i-s0] f16 layout
    msk = []
    for jc in range(TC):
        s0, fi = 128 * jc, FI[jc]
        m = np.zeros((128, G, fi), np.float16)
        sub = mm[s0:s0 + 128, s0:s0 + fi]
        for g in range(G):
            m[:, g, :] = (sub == g)
        msk.append(np.ascontiguousarray(m.reshape(128, G * fi)))
    w_qk = np.concatenate(
        [w_attn[:, hp * 64:(hp + 1) * 64],
         w_attn[:, C + hp * 64:C + (hp + 1) * 64]], axis=1
    ).astype(np.float32)                                            # (C, 128)
    w_v = np.ascontiguousarray(
        w_attn[:, 2 * C + hp * 64:2 * C + (hp + 1) * 64]).astype(np.float32)
    ektab = (eet @ w_edge_k)[:, hp * 64:(hp + 1) * 64] / np.sqrt(E)  # (G, 64)
    ektabT = np.ascontiguousarray(ektab.T).astype(np.float32)        # (64, G)
    evtab = (eet @ w_edge_v)[:, hp * 64:(hp + 1) * 64]               # (G, 64)
    evb = np.zeros((128, G * VW), np.float16)
    for g in range(G):
        evb[:, VW * g:VW * g + 32] = evtab[g, 0:32].astype(np.float16)[None, :]
        evb[:, VW * g + 32] = 1.0
        evb[:, VW * g + 33:VW * g + 65] = evtab[g, 32:64].astype(np.float16)[None, :]
        evb[:, VW * g + 65] = 1.0
    ebias = np.exp(abt)                                              # (G, H)
    cb = np.empty((2 * T, T), np.float16)
    for h in range(2):
        cb[h * T:(h + 1) * T, :] = ebias[:, h0 + h][bmT]
    w_proj_sl = np.ascontiguousarray(
        w_proj[hp * 64:(hp + 1) * 64, :]).astype(np.float16)         # (64, C)
    d = {
        "xT": xT, "cb": cb, "w_qk": w_qk, "w_v": w_v,
        "ektabT": ektabT, "evb": evb, "w_proj_sl": w_proj_sl,
    }
    for jc in range(TC):
        d[f"msk{jc}"] = msk[jc]
    return d


def run(inputs, trace=False):
    x = np.asarray(inputs["x"], np.float32)
    bm = np.asarray(inputs["bias_matrix"]).astype(np.int64)
    w_attn = np.asarray(inputs["w_attn"], np.float32)
    w_proj = np.asarray(inputs["w_proj"], np.float32)
    w_edge_k = np.asarray(inputs["w_edge_k"], np.float32)
    w_edge_v = np.asarray(inputs["w_edge_v"], np.float32)
    eet = np.asarray(inputs["edge_emb_table"], np.float32)
    abt = np.asarray(inputs["attn_bias_table"], np.float32)

    nc = _get_nc()
    in_maps = [
        _prep_core_inputs(c, x, bm, w_attn, w_proj, w_edge_k, w_edge_v, eet, abt)
        for c in range(NCORES)
    ]
    res = run_bass_kernel_spmd(nc, in_maps, core_ids=list(range(NCORES)),
                               trace=trace)
    out = np.zeros((B, T, C), np.float32)
    for c in range(NCORES):
        out[c // 4] += res.results[c]["out"]
    return out, res


def kernel(**inputs) -> np.ndarray:
    out, _ = run(inputs, trace=False)
    return out


# revision 13
# speedup vs baseline: 1.1057x; 1.0046x over previous
"""Edge-conditioned causal self-attention (Graphormer-style) on 8 Trainium2 cores.

Math (per batch b, head h; i = query pos, j = key pos; E=32, G=16 edge types):
  scores[i,j] = sum_e q[i,e] * k[j,e] * ektab[bm[j,i], e] / sqrt(E)
  attn        = softmax_j(scores + biastab[bm[i,j]])  (causal j<=i)
  y[i,e]      = sum_j attn[i,j] * v[j,e] * evtab[bm[j,i], e]
  out         = y @ w_proj
where ektab = edge_emb @ w_edge_k (per-head slice), evtab likewise, and bm is
the integer edge-type matrix.  NOTE: the additive bias indexes bm TRANSPOSED
relative to the k/v modulation (matches the reference einsums).

Since there are only G=16 edge types, scores decompose into 16 per-type Gram
planes S_g = (q .* ektab[g]) @ k^T.  The per-(j,i) plane selection is done by
masking in exp space: e2_g = mask_g * exp(S_g) * exp(bias), and the output
accumulates yT += (v .* evtab[g] | 1) @ e2_g with the ones column giving the
softmax denominator for free.

v2 structure (engine-balanced for the TimelineSim cost model):
  - S_g matmuls batched into multi-bank PSUM tiles; exp'd in batched strided
    activations on the Scalar engine (only engine with cheap PSUM exit).
  - mask multiply (e1): bottom NG g's per (h,jc) on GPSIMD via fused
    scalar_tensor_tensor (mm==g)*p (needs no mask planes); top NV g's on
    Vector via plain tensor_tensor against host-precomputed f16 mask planes
    (TT runs at 2x, STT only at 1x in the cost model).
  - bias multiply (e2): on Vector as wide TTs with stride-0-broadcast cb.
Sharding: core c -> batch b=c//4, head pair (2*(c%4), 2*(c%4)+1).  Fully
data-parallel, no collectives; host sums the 4 per-core partial projections
per batch (w_proj is row-parallel over head slices).
"""

import numpy as np

import concourse.bass as bass  # noqa: F401
import concourse.mybir as mybir
import concourse.tile as tile
from concourse import bacc
from concourse.bass_utils import run_bass_kernel_spmd

B, T, C, H, E, G = 2, 512, 256, 8, 32, 16
NCORES = 8
EP = E + 1      # 33: v block width per head (32 modulated v cols + ones col)
VW = 2 * EP     # 66: v block for both heads
TC = T // 128   # 4 row chunks

NG = 4          # bottom-NG g's per (h,jc) handled on GPSIMD (e1+e2 TTs)
FI = [512, 384, 256, 128]          # i-extent per j-chunk
PSTRIDE = [512, 512, 256, 128]     # psum plane stride per j-chunk
# per-jc PSUM batching: list of (g0, nplanes) with nplanes*PSTRIDE <= 1536
BATCHES = [
    [(0, 3), (3, 3), (6, 3), (9, 3), (12, 3), (15, 1)],
    [(0, 3), (3, 3), (6, 3), (9, 3), (12, 3), (15, 1)],
    [(0, 6), (6, 6), (12, 4)],
    [(0, 12), (12, 4)],
]

F16 = mybir.dt.float16
F32 = mybir.dt.float32
EXP = mybir.ActivationFunctionType.Exp
EQ = mybir.AluOpType.is_equal
MUL = mybir.AluOpType.mult

_NC_CACHE = [None]


def _build_nc():
    nc = bacc.Bacc("TRN2", target_bir_lowering=False)

    d_xT = nc.dram_tensor("xT", (C, T), F32, kind="ExternalInput")
    d_msk = [
        nc.dram_tensor(f"msk{jc}", (128, G * FI[jc]), F16, kind="ExternalInput")
        for jc in range(TC)
    ]
    d_cb = nc.dram_tensor("cb", (2 * T, T), F16, kind="ExternalInput")
    d_wqk = nc.dram_tensor("w_qk", (C, 128), F32, kind="ExternalInput")
    d_wv = nc.dram_tensor("w_v", (C, 64), F32, kind="ExternalInput")
    d_ekT = nc.dram_tensor("ektabT", (64, G), F32, kind="ExternalInput")
    d_evb = nc.dram_tensor("evb", (128, G * VW), F16, kind="ExternalInput")
    d_wp = nc.dram_tensor("w_proj_sl", (64, C), F16, kind="ExternalInput")
    d_out = nc.dram_tensor("out", (T, C), F32, kind="ExternalOutput")

    with tile.TileContext(nc) as tc:
        with (
            tc.tile_pool(name="const", bufs=1) as const,
            tc.tile_pool(name="ps_misc", bufs=1, space="PSUM") as ps_misc,
            tc.tile_pool(name="ps_tri", bufs=2, space="PSUM") as ps_tri,
            tc.tile_pool(name="ps_y", bufs=1, space="PSUM") as ps_y,
            tc.tile_pool(name="pp", bufs=3) as pp,
            tc.tile_pool(name="e1p", bufs=2) as e1p,
            tc.tile_pool(name="e2p", bufs=2) as e2p,
        ):
            # ---------------- input loads ----------------
            xT_s = []
            for i in range(2):
                t_ = const.tile([128, T], F32, name=f"xT{i}")
                nc.sync.dma_start(out=t_[:], in_=d_xT[128 * i:128 * (i + 1), :])
                xT_s.append(t_)
            wqk_s = []
            for i in range(2):
                t_ = const.tile([128, 128], F32, name=f"wqk{i}")
                nc.sync.dma_start(out=t_[:], in_=d_wqk[128 * i:128 * (i + 1), :])
                wqk_s.append(t_)
            wv_s = []
            for i in range(2):
                t_ = const.tile([128, 64], F32, name=f"wv{i}")
                nc.sync.dma_start(out=t_[:], in_=d_wv[128 * i:128 * (i + 1), :])
                wv_s.append(t_)
            ekT_s = const.tile([64, G], F32, name="ekT")
            nc.sync.dma_start(out=ekT_s[:], in_=d_ekT[:, :])
            # per-jc tiles, in processing order: masks (e1), cb (bias; h0 first)
            msk_s = []
            cb_s = [[None] * TC, [None] * TC]
            for jc in range(TC):
                t_ = const.tile([128, G * FI[jc]], F16, name=f"msk{jc}")
                nc.sync.dma_start(out=t_[:], in_=d_msk[jc][:, :])
                msk_s.append(t_)
                t_ = const.tile([128, T], F16, name=f"cb0_{jc}")
                nc.sync.dma_start(
                    out=t_[:], in_=d_cb[128 * jc:128 * (jc + 1), :])
                cb_s[0][jc] = t_
            evb_s = const.tile([128, G * VW], F16, name="evb")
            nc.sync.dma_start(out=evb_s[:], in_=d_evb[:, :])
            for jc in range(TC):
                t_ = const.tile([128, T], F16, name=f"cb1_{jc}")
                nc.sync.dma_start(
                    out=t_[:], in_=d_cb[T + 128 * jc:T + 128 * (jc + 1), :])
                cb_s[1][jc] = t_
            wp_s = const.tile([64, C], F16, name="wp")
            nc.sync.dma_start(out=wp_s[:], in_=d_wp[:, :])

            # ---------------- q/k/v projections ----------------
            # qk_ps rows: 0-31 qT(h0), 32-63 qT(h1), 64-95 kT(h0), 96-127 kT(h1)
            qk_ps = ps_misc.tile([128, T], F32, tag="misc", name="qk_ps")
            for ck in range(2):
                nc.tensor.matmul(
                    qk_ps[:],
                    lhsT=wqk_s[ck][:],
                    rhs=xT_s[ck][:],
                    start=(ck == 0), stop=(ck == 1),
                )
            qk_sb = const.tile([128, T], F16, name="qk_sb")
            nc.vector.tensor_copy(out=qk_sb[:], in_=qk_ps[:])

            # v_ps cols per t-chunk: 32 v(h0) | 32 v(h1)
            v_ps = ps_misc.tile([128, TC * 64], F32, tag="misc", name="v_ps")
            for tcc in range(TC):
                for ck in range(2):
                    nc.tensor.matmul(
                        v_ps[:, 64 * tcc:64 * (tcc + 1)],
                        lhsT=xT_s[ck][:, 128 * tcc:128 * (tcc + 1)],
                        rhs=wv_s[ck][:],
                        start=(ck == 0), stop=(ck == 1),
                    )

            # k tiles (contraction layout: e on partitions, j on free)
            k_h = []
            for h in range(2):
                t_ = const.tile([E, T], F16, name=f"k{h}")
                nc.vector.tensor_copy(out=t_[:], in_=qk_sb[64 + 32 * h:96 + 32 * h, :])
                k_h.append(t_)

            # q' tiles: per g block of T cols, rows = e, modulated by ektab/sqrt(E)
            q_all = []
            for h in range(2):
                t_ = const.tile([E, G * T], F16, name=f"qall{h}")
                for g in range(G):
                    nc.vector.tensor_scalar_mul(
                        t_[:, T * g:T * (g + 1)],
                        qk_sb[32 * h:32 * h + 32, :],
                        ekT_s[32 * h:32 * h + 32, g:g + 1],
                    )
                q_all.append(t_)

            # v66 per t-chunk: [v_h0(32) | 1 | v_h1(32) | 1]
            v66 = []
            for tcc in range(TC):
                t_ = const.tile([128, VW], F16, name=f"v66_{tcc}")
                nc.vector.tensor_copy(out=t_[:, 0:32], in_=v_ps[:, 64 * tcc:64 * tcc + 32])
                nc.vector.tensor_copy(out=t_[:, 33:65], in_=v_ps[:, 64 * tcc + 32:64 * tcc + 64])
                nc.vector.memset(t_[:, 32:33], 1.0)
                nc.vector.memset(t_[:, 65:66], 1.0)
                v66.append(t_)

            # v'_g = v66 * evtab broadcast: one wide TT per t-chunk
            vall = []
            for tcc in range(TC):
                t_ = const.tile([128, G * VW], F16, name=f"vall{tcc}")
                nc.vector.tensor_mul(
                    t_[:].rearrange("p (g w) -> p g w", g=G),
                    v66[tcc][:].unsqueeze(1).broadcast_to([128, G, VW]),
                    evb_s[:].rearrange("p (g w) -> p g w", g=G),
                )
                vall.append(t_)

            ones32 = const.tile([1, 32], F32, name="ones32")
            nc.vector.memset(ones32[:], 1.0)
            yTn = const.tile([64, T], F16, name="yTn")

            # ---------------- main attention loops ----------------
            # Software-pipelined emission: each batch's yT matmuls are
            # emitted LAG batches late so the in-order PE sequencer never
            # queues a dependency-blocked yT matmul ahead of ready S matmuls
            # (PE wait-queue depth is 4; head-of-line blocking otherwise
            # serializes the whole pipeline).
            LAG = 2
            yT_h = [None, None]
            e1t_g = e2t_g = None
            cur_group = None

            flat = []
            for h in range(2):
                for jc in range(TC):
                    for bi, (g0, np_) in enumerate(BATCHES[jc]):
                        flat.append((h, jc, g0, np_,
                                     bi == len(BATCHES[jc]) - 1 and jc == TC - 1))

            def emit_norm(h):
                yT = yT_h[h]
                recip = const.tile([1, T], F32, name=f"recip{h}")
                nc.vector.reciprocal(recip[:], yT[32:33, :])
                bc_ps = ps_misc.tile([32, T], F32, tag="misc", name=f"bc{h}")
                nc.tensor.matmul(bc_ps[:], lhsT=ones32[:], rhs=recip[:],
                                 start=True, stop=True)
                rb_sb = const.tile([32, T], F32, name=f"rb{h}")
                nc.scalar.copy(rb_sb[:], bc_ps[:])
                nc.vector.tensor_mul(yTn[32 * h:32 * (h + 1), :], yT[0:32, :],
                                     rb_sb[:])

            first_h = [True, True]

            def emit_yt(item):
                h, jc, g0, np_, last, e2t = item
                fi = FI[jc]
                s0 = 128 * jc
                for k in range(np_):
                    g = g0 + k
                    nc.tensor.matmul(
                        yT_h[h][:, s0:s0 + fi],
                        lhsT=vall[jc][:, VW * g + EP * h:VW * g + EP * (h + 1)],
                        rhs=e2t[:, g * fi:(g + 1) * fi],
                        start=first_h[h],
                        stop=(last and k == np_ - 1),
                    )
                    first_h[h] = False
                if last:
                    emit_norm(h)

            pending = []
            for (h, jc, g0, np_, last) in flat:
                s0 = 128 * jc
                fi = FI[jc]
                ps = PSTRIDE[jc]
                ghi = g0 + np_
                if yT_h[h] is None:
                    yT_h[h] = ps_y.tile([EP, T], F32, tag="y", name=f"yT{h}")
                if cur_group != (h, jc):
                    cur_group = (h, jc)
                    e1t_g = e1p.tile([128, G * fi], F16, tag="e1", name="e1")
                    e2t_g = e2p.tile([128, G * fi], F16, tag="e2", name="e2")
                e1t, e2t = e1t_g, e2t_g

                tri = ps_tri.tile([128, 1536], F32, tag="tri", name="tri")
                for k in range(np_):
                    g = g0 + k
                    nc.tensor.matmul(
                        tri[:, ps * k:ps * k + fi],
                        lhsT=k_h[h][:, s0:s0 + 128],
                        rhs=q_all[h][:, T * g + s0:T * (g + 1)],
                        start=True, stop=True,
                    )
                p_ = pp.tile([128, 1536], F16, tag="p", name="p")
                if ps == fi:
                    nc.scalar.activation(
                        p_[:, 0:np_ * fi], tri[:, 0:np_ * fi], EXP)
                else:
                    nc.scalar.activation(
                        p_[:, 0:np_ * ps].rearrange(
                            "p (k s) -> p k s", k=np_)[:, :, 0:fi],
                        tri[:, 0:np_ * ps].rearrange(
                            "p (k s) -> p k s", k=np_)[:, :, 0:fi],
                        EXP,
                    )

                # e1 = mask * p; e2 = e1 * cb.  g < NG on gpsimd (per-plane
                # TTs), g >= NG on vector (wide TTs).
                for k in range(np_):
                    g = g0 + k
                    if g >= NG:
                        break
                    nc.gpsimd.tensor_mul(
                        e1t[:, g * fi:(g + 1) * fi],
                        msk_s[jc][:, g * fi:(g + 1) * fi],
                        p_[:, ps * k:ps * k + fi],
                    )
                    nc.gpsimd.tensor_mul(
                        e2t[:, g * fi:(g + 1) * fi],
                        e1t[:, g * fi:(g + 1) * fi],
                        cb_s[h][jc][:, s0:s0 + fi],
                    )
                glo = max(g0, NG)
                if ghi > glo:
                    nrun = ghi - glo
                    koff = glo - g0
                    psrc = p_[:, ps * koff:ps * (koff + nrun)].rearrange(
                        "p (k s) -> p k s", k=nrun)[:, :, 0:fi]
                    nc.vector.tensor_mul(
                        e1t[:, glo * fi:ghi * fi].rearrange(
                            "p (k s) -> p k s", k=nrun),
                        msk_s[jc][:, glo * fi:ghi * fi].rearrange(
                            "p (k s) -> p k s", k=nrun),
                        psrc,
                    )
                    nc.vector.tensor_mul(
                        e2t[:, glo * fi:ghi * fi].rearrange(
                            "p (k s) -> p k s", k=nrun),
                        e1t[:, glo * fi:ghi * fi].rearrange(
                            "p (k s) -> p k s", k=nrun),
                        cb_s[h][jc][:, s0:s0 + fi].unsqueeze(1).broadcast_to(
                            [128, nrun, fi]),
                    )

                pending.append((h, jc, g0, np_, last, e2t))
                if len(pending) > LAG:
                    emit_yt(pending.pop(0))
            for item in pending:
                emit_yt(item)

            # ---------------- output projection ----------------
            for tcc in range(TC):
                o_ps = ps_misc.tile([128, C], F32, tag="misc", name=f"o_ps{tcc}")
                nc.tensor.matmul(
                    o_ps[:],
                    lhsT=yTn[:, 128 * tcc:128 * (tcc + 1)],
                    rhs=wp_s[:],
                    start=True, stop=True,
                )
                o_sb = const.tile([128, C], F32, name=f"o_sb{tcc}")
                nc.scalar.copy(o_sb[:], o_ps[:])
                nc.sync.dma_start(out=d_out[128 * tcc:128 * (tcc + 1), :], in_=o_sb[:])

    nc.compile()
    return nc


def _get_nc():
    if _NC_CACHE[0] is None:
        _NC_CACHE[0] = _build_nc()
    return _NC_CACHE[0]


def _prep_core_inputs(c, x, bm, w_attn, w_proj, w_edge_k, w_edge_v, eet, abt):
    b, hp = divmod(c, 4)
    h0 = 2 * hp
    xT = np.ascontiguousarray(x[b].T).astype(np.float32)            # (C, T)
    triu = np.triu(np.ones((T, T), dtype=bool))                     # j <= i
    bmT = bm[b].T                                                   # bmT[j,i] = bm[b][i,j]
    mm = np.where(triu, bm[b], 255)                                 # (T, T) [j,i]
    # mask planes for all G edge types, per j-chunk, [j, g, i-s0] f16 layout
    msk = []
    for jc in range(TC):
        s0, fi = 128 * jc, FI[jc]
        m = np.zeros((128, G, fi), np.float16)
        sub = mm[s0:s0 + 128, s0:s0 + fi]
        for g in range(G):
            m[:, g, :] = (sub == g)
        msk.append(np.ascontiguousarray(m.reshape(128, G * fi)))
    w_qk = np.concatenate(
        [w_attn[:, hp * 64:(hp + 1) * 64],
         w_attn[:, C + hp * 64:C + (hp + 1) * 64]], axis=1
    ).astype(np.float32)                                            # (C, 128)
    w_v = np.ascontiguousarray(
        w_attn[:, 2 * C + hp * 64:2 * C + (hp + 1) * 64]).astype(np.float32)
    ektab = (eet @ w_edge_k)[:, hp * 64:(hp + 1) * 64] / np.sqrt(E)  # (G, 64)
    ektabT = np.ascontiguousarray(ektab.T).astype(np.float32)        # (64, G)
    evtab = (eet @ w_edge_v)[:, hp * 64:(hp + 1) * 64]               # (G, 64)
    evb = np.zeros((128, G * VW), np.float16)
    for g in range(G):
        evb[:, VW * g:VW * g + 32] = evtab[g, 0:32].astype(np.float16)[None, :]
        evb[:, VW * g + 32] = 1.0
        evb[:, VW * g + 33:VW * g + 65] = evtab[g, 32:64].astype(np.float16)[None, :]
        evb[:, VW * g + 65] = 1.0
    ebias = np.exp(abt)                                              # (G, H)
    cb = np.empty((2 * T, T), np.float16)
    for h in range(2):
        cb[h * T:(h + 1) * T, :] = ebias[:, h0 + h][bmT]
    w_proj_sl = np.ascontiguousarray(
        w_proj[hp * 64:(hp + 1) * 64, :]).astype(np.float16)         # (64, C)
    d = {
        "xT": xT, "cb": cb, "w_qk": w_qk, "w_v": w_v,
        "ektabT": ektabT, "evb": evb, "w_proj_sl": w_proj_sl,
    }
    for jc in range(TC):
        d[f"msk{jc}"] = msk[jc]
    return d


def run(inputs, trace=False):
    x = np.asarray(inputs["x"], np.float32)
    bm = np.asarray(inputs["bias_matrix"]).astype(np.int64)
    w_attn = np.asarray(inputs["w_attn"], np.float32)
    w_proj = np.asarray(inputs["w_proj"], np.float32)
    w_edge_k = np.asarray(inputs["w_edge_k"], np.float32)
    w_edge_v = np.asarray(inputs["w_edge_v"], np.float32)
    eet = np.asarray(inputs["edge_emb_table"], np.float32)
    abt = np.asarray(inputs["attn_bias_table"], np.float32)

    nc = _get_nc()
    in_maps = [
        _prep_core_inputs(c, x, bm, w_attn, w_proj, w_edge_k, w_edge_v, eet, abt)
        for c in range(NCORES)
    ]
    res = run_bass_kernel_spmd(nc, in_maps, core_ids=list(range(NCORES)),
                               trace=trace)
    out = np.zeros((B, T, C), np.float32)
    for c in range(NCORES):
        out[c // 4] += res.results[c]["out"]
    return out, res


def kernel(**inputs) -> np.ndarray:
    out, _ = run(inputs, trace=False)
    return out
